# revision 63
# baseline (speedup 1.0000x reference)
"""CARAFE content-aware upsampling kernel for Trainium2 (8 NeuronCores).

Problem: x(4,256,64,64) -> 1x1 down-conv(64ch) -> 3x3 enc-conv(100ch) ->
softmax over 25 reassembly taps -> content-aware reassembly + pixel shuffle
(x2) -> 1x1 out-conv(256ch).  Output (4,256,128,128).

Sharding: data-parallel over (batch n, H-half) = 8 shards; each core computes
32 output rows (64 upsampled rows) of one image.

Per-core algorithm (v4 — fp16 matmuls, minimal DMA count, dy-pair packing):
  A) t = W_down@x + b_down          (64, 34, 68)  channels-on-partitions
  B) e = conv3x3(t) + b_enc         (100, 32*64)  via 9 shifted fp16 matmuls,
     PE-transpose (w-interleaved) -> softmax over 25 taps -> kern fp16 with
     partitions (w*2 + row-parity); 6 plain-slice DMAs per chunk build the
     partition-shifted S3 operand for phase D's scatters.
  C) y0 = W_out@x (bias added post-reassembly; exact because the softmax
     weights sum to 1 and zero-padded x gives y0=0 at pad positions).
     Stationary = xs[:, g:g+2, 2:66] so PSUM partitions come out as
     (row-offset dr, col w') = the layout phase D needs (YS2).  Rows are
     emitted interleaved with phase D to keep PE warm while scatters run.
  D) reassembly per output row h: one gpsimd local_scatter builds a banded
     fp16 matrix Bc[(dr,w'), (slot,i,w,jj)] packing dy-pairs {0,1},{2,3} into
     128-partition contractions plus a 64-partition dy=4 tile -> 3
     PSUM-accumulated matmuls per c-half.  b_out is added during the
     PSUM->SBUF copy (DVE for c-half 0, Act for c-half 1), 4 rows per output
     DMA (cf0 on the SP queue, cf1 on the Act queue).
"""
import sys

for _p in ("/opt/trn_rl_repo",):
    if _p not in sys.path:
        sys.path.insert(0, _p)

import numpy as np

N, C, H, W = 4, 256, 64, 64
D, KUP = 2, 5
CM, E, OC = 64, 100, 256
HH = 32          # output rows per core
RS = 37          # x slab rows (2-halo each side + 1 pad row for phase C pairs)
TR = HH + 2      # t rows (1-halo each side)
WP = W + 4       # padded width

_CACHE = {}

# per-j valid-w windows for the S3 partition-shifted copies:
# S3[q, par, s, j*100+ch] = kern[2*w + par, s, ch] with w = q%64 + j - 2
_JRANGES = [(0, 62, 2), (0, 63, 1), (0, 64, 0), (1, 63, 0), (2, 62, 0)]

# x slab DMA row chunks (phase A starts once the first chunk lands)
_XCHUNKS = ((0, 8), (8, 22), (22, RS))


def _scatter_index_table() -> np.ndarray:
    """si3[q, j*100+ch] -> column in banded Bc[128, 768].

    Partition q = dr*64 + w' (dr = dy-pair row offset, w' = y column).
    Bc columns: slot*256 + i*128 + w*2 + jj, slot 0 = dy{0,1}, slot 1 =
    dy{2,3}, slot 2 = dy 4 (dr=0 partitions only).
    """
    si3 = np.full((128, 512), -1, np.int16)
    for q in range(128):
        dr, wpp = q // 64, q % 64
        for j in range(5):
            w = wpp + j - 2
            if not (0 <= w < W):
                continue
            dxi = 4 - j
            for dy in range(5):
                if dy == 4:
                    if dr != 0:
                        continue
                    slot = 2
                elif dy % 2 == dr:
                    slot = (dy - dr) // 2
                else:
                    continue
                for p in range(4):
                    i, jj = p // 2, p % 2
                    ch = p * 25 + dy * 5 + dxi  # p-major enc channels
                    si3[q, j * E + ch] = slot * 256 + i * 128 + w * 2 + jj
    return si3


def _build_program():
    if "nc" in _CACHE:
        return _CACHE["nc"]

    import concourse.bacc as bacc
    import concourse.mybir as mybir
    import concourse.tile as tile
    from concourse import bass

    F32, F16, I16 = mybir.dt.float32, mybir.dt.float16, mybir.dt.int16
    PSUM = bass.MemorySpace.PSUM
    Act = mybir.ActivationFunctionType

    nc = bacc.Bacc("TRN2", target_bir_lowering=False, debug=False, num_devices=8)

    xs_d = nc.dram_tensor("xs", [2, 128, RS, WP], F16, kind="ExternalInput")
    xp_d = nc.dram_tensor("xp", [2, 128, RS - 1, 128], F16, kind="ExternalInput")
    ba_d = nc.dram_tensor("blobA", [128, 2 * CM], F16, kind="ExternalInput")
    sa_d = nc.dram_tensor("smallA", [1, CM + RS * WP], F16, kind="ExternalInput")
    bw_d = nc.dram_tensor("blobW", [128, 900], F16, kind="ExternalInput")
    bo_d = nc.dram_tensor("blobO", [128, 2 * OC], F16, kind="ExternalInput")
    bc_d = nc.dram_tensor("blobC", [128, 130], F32, kind="ExternalInput")
    si_d = nc.dram_tensor("six", [128, 512], I16, kind="ExternalInput")
    out_d = nc.dram_tensor("out", [2, 128, HH, 2, 128], F32, kind="ExternalOutput")

    with tile.TileContext(nc) as tc:
        with (
            tc.tile_pool(name="const", bufs=1) as cp,
            tc.tile_pool(name="esb", bufs=2) as ep_sb,
            tc.tile_pool(name="sm", bufs=2) as smp,
            tc.tile_pool(name="sB", bufs=6) as bp,
            tc.tile_pool(name="ro", bufs=4) as rop,
        ):
            xs0 = cp.tile([128, RS, WP], F16, tag="xs0")
            xs1 = cp.tile([128, RS, WP], F16, tag="xs1")
            xp0 = cp.tile([128, RS - 1, 128], F16, tag="xp0")
            xp1 = cp.tile([128, RS - 1, 128], F16, tag="xp1")
            ba_t = cp.tile([128, 2 * CM], F16, tag="blobA")
            sa_t = cp.tile([1, CM + RS * WP], F16, tag="smallA")
            bw_t = cp.tile([128, 900], F16, tag="blobW")
            bo_t = cp.tile([128, 2 * OC], F16, tag="blobO")
            bc_t = cp.tile([128, 130], F32, tag="blobC")
            si_t = cp.tile([128, 512], I16, tag="six")
            t_t = cp.tile([CM + 1, TR, WP], F16, tag="t")
            kern = cp.tile([128, 16, E], F16, tag="kern")
            S3 = cp.tile([128, 2, 16, 512], F16, tag="S3")
            YS2 = cp.tile([128, RS, OC], F16, tag="YS2")

            wd0, wd1 = ba_t[:, 0:CM], ba_t[:, CM : 2 * CM]
            bd_v = sa_t[:, 0:CM]
            vm_v = sa_t[:, CM:].rearrange("p (r w) -> p r w", r=RS)
            we_v = bw_t[0 : CM + 1, :].rearrange("p (t e) -> p t e", t=9)
            wo0, wo1 = bo_t[:, 0:OC], bo_t[:, OC : 2 * OC]
            id_v = bc_t[0:E, 0:E]
            bo0, bo1 = bc_t[:, 128:129], bc_t[:, 129:130]

            # SP queue: phase-A inputs first (x slab in 3 row chunks so phase
            # A starts as soon as the first rows land).  Act queue: only the
            # immediately-needed weights early — si/wo follow the first conv
            # chunk so their transfers don't delay the x slab.
            nc.sync.dma_start(ba_t[:], ba_d[:])
            nc.sync.dma_start(sa_t[:], sa_d[:])
            r0, r1 = _XCHUNKS[0]
            nc.sync.dma_start(xs0[:, r0:r1, :], xs_d[0, :, r0:r1, :])
            nc.sync.dma_start(xs1[:, r0:r1, :], xs_d[1, :, r0:r1, :])
            nc.sync.dma_start(bw_t[:], bw_d[:])
            nc.sync.dma_start(bc_t[:], bc_d[:])
            nc.sync.dma_start(si_t[:], si_d[:])
            for r0, r1 in _XCHUNKS[1:]:
                nc.sync.dma_start(xs0[:, r0:r1, :], xs_d[0, :, r0:r1, :])
                nc.sync.dma_start(xs1[:, r0:r1, :], xs_d[1, :, r0:r1, :])
            nc.sync.dma_start(bo_t[:], bo_d[:])
            nc.vector.memset(t_t[CM : CM + 1, :, :], 1.0)
            # zero-fill S3 once on the (otherwise idle) Pool engine so the
            # j-range edge cells the scatters read are defined; split in two
            # so neither S3 batch waits on the later fill
            nc.gpsimd.memset(S3[:, :, 0:6, :], 0.0)
            nc.gpsimd.memset(S3[:, :, 6:16, :], 0.0)

            # ---- phases A+B interleaved: B chunk k needs only A chunks
            # <= k+1, so emitting A0,A1,B0,A2,B1,... gets kern chunk 0 (and
            # with it the phase-D scatter chain) started ~7us earlier than
            # a strict A-then-B order.
            with (
                tc.tile_pool(name="tp", bufs=2, space=PSUM) as tpp,
                tc.tile_pool(name="ep", bufs=2, space=PSUM) as epp,
                tc.tile_pool(name="etp", bufs=2, space=PSUM) as etpp,
            ):
                def a_chunk(r0):
                    nr = min(7, TR - r0)
                    tp = tpp.tile([CM, nr, WP], F32, tag="tp", name="tp")
                    nc.tensor.matmul(tp[:], wd0, xs0[:, 1 + r0 : 1 + r0 + nr, :],
                                     start=True, stop=False)
                    nc.tensor.matmul(tp[:], wd1, xs1[:, 1 + r0 : 1 + r0 + nr, :],
                                     start=False, stop=False)
                    nc.tensor.matmul(tp[:], bd_v, vm_v[:, 1 + r0 : 1 + r0 + nr, :],
                                     start=False, stop=True)
                    nc.vector.tensor_copy(t_t[0:CM, r0 : r0 + nr, :], tp[:])

                def b_chunk(r0, nr, s0, ns):
                    ep = epp.tile([E, nr, W], F32, tag="ep", name="ep")
                    for tap in range(9):
                        dy, dx = tap // 3, tap % 3
                        nc.tensor.matmul(
                            ep[:],
                            we_v[:, tap, :],
                            t_t[:, r0 + dy : r0 + dy + nr, 1 + dx : 1 + dx + W],
                            start=(tap == 0), stop=(tap == 8),
                        )
                    es = ep_sb.tile([E, nr, W], F32, tag="es", name="es")
                    nc.scalar.activation(es[:], ep[:], Act.Copy)
                    for s in range(ns):
                        etp = etpp.tile([128, E], F32, tag="etp", name="etp")
                        nc.tensor.transpose(etp[:], es[:, 2 * s : 2 * s + 2, :],
                                            id_v)
                        slot = kern[:, s0 + s, :]
                        nc.scalar.activation(slot, etp[:], Act.Exp)
                        kv = slot.rearrange("p (q k) -> p q k", q=4)
                        ssum = smp.tile([128, 4, 1], F32, tag="ssum", name="ssum")
                        nc.vector.tensor_reduce(ssum[:], kv, mybir.AxisListType.X,
                                                mybir.AluOpType.add)
                        rinv = smp.tile([128, 4, 1], F32, tag="rinv", name="rinv")
                        nc.vector.reciprocal(rinv[:], ssum[:])
                        nc.vector.tensor_tensor(kv, kv, rinv[:].to_broadcast([128, 4, 25]),
                                                mybir.AluOpType.mult)
                def s3_batch(s0, ns):
                    # S3 fill for slots [s0, s0+ns): 5 partition-shifted kern
                    # copies + 1 dr-duplicate per parity, all on the SP queue
                    # (parity 0 first — it gates the even output rows).
                    for par in range(2):
                        for j in range(5):
                            w0, cnt, q0 = _JRANGES[j]
                            nc.sync.dma_start(
                                S3[q0 : q0 + cnt, par, s0 : s0 + ns,
                                   j * E : j * E + E],
                                kern[64 * par + w0 : 64 * par + w0 + cnt,
                                     s0 : s0 + ns, :],
                            )
                        nc.sync.dma_start(S3[64:128, par, s0 : s0 + ns, :],
                                          S3[0:64, par, s0 : s0 + ns, :])

                a_chunk(0)
                b_chunk(0, 4, 0, 2)
                nc.scalar.dma_start(xp0[:], xp_d[0])
                nc.scalar.dma_start(xp1[:], xp_d[1])
                a_chunk(7)
                b_chunk(4, 8, 2, 4)
                s3_batch(0, 6)
                a_chunk(14)
                a_chunk(21)
                b_chunk(12, 8, 6, 4)
                a_chunk(28)
                b_chunk(20, 8, 10, 4)
                b_chunk(28, 4, 14, 2)
                s3_batch(6, 10)

            # ---- phases C+D interleaved ----
            # C: YS2[(dr,w'), g] = y0[row g-2+dr, col w'] fp16; rows beyond
            # g=4 are emitted inside the D loop (D row h needs g <= h+4).
            # D: banded reassembly, 3 matmuls per (h, c-half).
            with (
                tc.tile_pool(name="yp", bufs=2, space=PSUM) as ypp,
                tc.tile_pool(name="rp", bufs=4, space=PSUM) as rpp,
            ):
                def c_row(g):
                    yp = ypp.tile([128, OC], F32, tag="yp", name="yp")
                    nc.tensor.matmul(yp[:], xp0[:, g, :], wo0,
                                     start=True, stop=False)
                    nc.tensor.matmul(yp[:], xp1[:, g, :], wo1,
                                     start=False, stop=True)
                    nc.scalar.activation(YS2[:, g, :], yp[:], Act.Copy)

                for g in range(5):
                    c_row(g)
                # process rows even-ahead (0, 2, 1, 4, 3, ...): even rows are
                # gated only on the parity-0 S3 stream, keeping Pool busy
                # while each batch's parity-1 DMAs land.
                OB = 8          # output rows per DMA batch
                order = [0] + [x for k in range(1, HH // 2)
                               for x in (2 * k, 2 * k - 1)] + [HH - 1]
                robs = {}
                done = [0] * (HH // OB)
                next_c = 5
                for h in order:
                    b0 = h - h % OB
                    if b0 not in robs:
                        robs[b0] = (
                            rop.tile([128, OB, 2, 128], F32, tag="rob0",
                                     name="rob0"),
                            rop.tile([128, OB, 2, 128], F32, tag="rob1",
                                     name="rob1"),
                        )
                    rob = robs[b0]
                    Bc = bp.tile([128, 768], F16, tag="Bc")
                    nc.gpsimd.local_scatter(Bc[:], S3[:, h % 2, h // 2, :], si_t[:],
                                            channels=128, num_elems=768, num_idxs=512)
                    while next_c <= min(h + 6, RS - 2):
                        c_row(next_c)
                        next_c += 1
                    for cf in range(2):
                        rp = rpp.tile([128, 2, 128], F32, tag="rp")
                        nc.tensor.matmul(rp[:], YS2[:, h, 128 * cf : 128 * (cf + 1)],
                                         Bc[:, 0:256], start=True, stop=False)
                        nc.tensor.matmul(rp[:], YS2[:, h + 2, 128 * cf : 128 * (cf + 1)],
                                         Bc[:, 256:512], start=False, stop=False)
                        nc.tensor.matmul(rp[:], YS2[0:64, h + 4, 128 * cf : 128 * (cf + 1)],
                                         Bc[0:64, 512:768], start=False, stop=True)
                        dst = rob[cf][:, h % OB, :, :]
                        if cf == 0:
                            nc.vector.tensor_tensor(dst, rp[:],
                                                    bo0.to_broadcast([128, 2, 128]),
                                                    mybir.AluOpType.add)
                        else:
                            nc.scalar.activation(dst, rp[:], Act.Identity,
                                                 bias=bo1)
                    done[b0 // OB] += 1
                    if done[b0 // OB] == OB:
                        nc.scalar.dma_start(out_d[0, :, b0 : b0 + OB, :, :],
                                            rob[0][:])
                        nc.scalar.dma_start(out_d[1, :, b0 : b0 + OB, :, :],
                                            rob[1][:])
                        del robs[b0]

    nc.compile()
    _CACHE["nc"] = nc
    return nc


def _host_inputs(x, W_down, b_down, W_enc, b_enc, W_out, b_out):
    """Per-core input maps (core = 2*n + h_half)."""
    blobA = np.ascontiguousarray(
        W_down.T.reshape(2, 128, CM).transpose(1, 0, 2).reshape(128, 2 * CM),
        np.float16)
    # p-major enc-channel permutation: ch' = p*25 + k  (orig ch = k*4 + p)
    perm = np.array([k * 4 + p for p in range(4) for k in range(25)])
    we = np.zeros((128, 9, E), np.float16)
    for tap in range(9):
        dy, dx = tap // 3, tap % 3
        we[:CM, tap, :] = W_enc[perm, :, dy, dx].T.astype(np.float16)
    we[CM, 4, :] = b_enc[perm].astype(np.float16)
    blobW = we.reshape(128, 900)
    blobO = np.ascontiguousarray(
        W_out.T.reshape(2, 128, OC).transpose(1, 0, 2).reshape(128, 2 * OC),
        np.float16)
    blobC = np.concatenate(
        [np.eye(128, dtype=np.float32), b_out.reshape(2, 128).T.astype(np.float32)],
        axis=1)
    six = _scatter_index_table()

    in_maps = []
    for core in range(8):
        n, h0 = core // 2, (core % 2) * HH
        xs = np.zeros((C, RS, WP), np.float16)
        vm = np.zeros((RS, WP), np.float16)
        lo, hi = max(0, h0 - 2), min(H, h0 + HH + 2)
        xs[:, lo - (h0 - 2) : hi - (h0 - 2), 2 : 2 + W] = x[n, :, lo:hi, :]
        vm[lo - (h0 - 2) : hi - (h0 - 2), 2 : 2 + W] = 1.0
        smallA = np.concatenate(
            [b_down.astype(np.float16), vm.reshape(-1)])[None, :].astype(np.float16)
        # xp: phase-C stationary pairs xp[c, g, rr*64+w] = xs[c, g+rr, 2+w]
        sl = xs[:, :, 2 : 2 + W]
        xp = np.ascontiguousarray(
            np.lib.stride_tricks.sliding_window_view(sl, 2, axis=1)
            .transpose(0, 1, 3, 2).reshape(C, RS - 1, 128), np.float16)
        in_maps.append({
            "xs": xs.reshape(2, 128, RS, WP),
            "xp": xp.reshape(2, 128, RS - 1, 128),
            "blobA": blobA, "smallA": smallA, "blobW": blobW, "blobO": blobO,
            "blobC": blobC, "six": six,
        })
    return in_maps


def kernel(x, W_down, b_down, W_enc, b_enc, W_out, b_out):
    from concourse.bass_utils import run_bass_kernel_spmd

    nc = _build_program()
    in_maps = _host_inputs(np.asarray(x, np.float32), np.asarray(W_down, np.float32),
                           np.asarray(b_down, np.float32), np.asarray(W_enc, np.float32),
                           np.asarray(b_enc, np.float32), np.asarray(W_out, np.float32),
                           np.asarray(b_out, np.float32))
    res = run_bass_kernel_spmd(nc, in_maps, list(range(8)))
    full = np.empty((N, C, 2 * H, 2 * W), np.float32)
    for core in range(8):
        n, half = core // 2, core % 2
        arr = res.results[core]["out"].reshape(C, HH * 2, 2 * W)
        full[n, :, half * 64 : (half + 1) * 64, :] = arr
    return full


# revision 73
# speedup vs baseline: 1.0242x; 1.0242x over previous
"""CARAFE content-aware upsampling kernel for Trainium2 (8 NeuronCores).

Problem: x(4,256,64,64) -> 1x1 down-conv(64ch) -> 3x3 enc-conv(100ch) ->
softmax over 25 reassembly taps -> content-aware reassembly + pixel shuffle
(x2) -> 1x1 out-conv(256ch).  Output (4,256,128,128).

Sharding: data-parallel over (batch n, H-half) = 8 shards; each core computes
32 output rows (64 upsampled rows) of one image.

Per-core algorithm (v4 — fp16 matmuls, minimal DMA count, dy-pair packing):
  A) t = W_down@x + b_down          (64, 34, 68)  channels-on-partitions
  B) e = conv3x3(t) + b_enc         (100, 32*64)  via 9 shifted fp16 matmuls,
     PE-transpose (w-interleaved) -> softmax over 25 taps -> kern fp16 with
     partitions (w*2 + row-parity); 6 plain-slice DMAs per chunk build the
     partition-shifted S3 operand for phase D's scatters.
  C) y0 = W_out@x (bias added post-reassembly; exact because the softmax
     weights sum to 1 and zero-padded x gives y0=0 at pad positions).
     Stationary = xs[:, g:g+2, 2:66] so PSUM partitions come out as
     (row-offset dr, col w') = the layout phase D needs (YS2).  Rows are
     emitted interleaved with phase D to keep PE warm while scatters run.
  D) reassembly per output row h: one gpsimd local_scatter builds a banded
     fp16 matrix Bc[(dr,w'), (slot,i,w,jj)] packing dy-pairs {0,1},{2,3} into
     128-partition contractions plus a 64-partition dy=4 tile -> 3
     PSUM-accumulated matmuls per c-half.  b_out is added during the
     PSUM->SBUF copy (DVE for c-half 0, Act for c-half 1), 4 rows per output
     DMA (cf0 on the SP queue, cf1 on the Act queue).
"""
import sys

for _p in ("/opt/trn_rl_repo",):
    if _p not in sys.path:
        sys.path.insert(0, _p)

import numpy as np

N, C, H, W = 4, 256, 64, 64
D, KUP = 2, 5
CM, E, OC = 64, 100, 256
HH = 32          # output rows per core
RS = 37          # x slab rows (2-halo each side + 1 pad row for phase C pairs)
TR = HH + 2      # t rows (1-halo each side)
WP = W + 4       # padded width

_CACHE = {}

# per-j valid-w windows for the S3 partition-shifted copies:
# S3[q, par, s, j*100+ch] = kern[2*w + par, s, ch] with w = q%64 + j - 2
_JRANGES = [(0, 62, 2), (0, 63, 1), (0, 64, 0), (1, 63, 0), (2, 62, 0)]

# x slab DMA row chunks (phase A starts once the first chunk lands)
_XCHUNKS = ((0, 8), (8, 22), (22, RS))


def _scatter_index_table() -> np.ndarray:
    """si3[q, j*100+ch] -> column in banded Bc[128, 768].

    Partition q = dr*64 + w' (dr = dy-pair row offset, w' = y column).
    Bc columns: slot*256 + i*128 + w*2 + jj, slot 0 = dy{0,1}, slot 1 =
    dy{2,3}, slot 2 = dy 4 (dr=0 partitions only).
    """
    si3 = np.full((128, 512), -1, np.int16)
    for q in range(128):
        dr, wpp = q // 64, q % 64
        for j in range(5):
            w = wpp + j - 2
            if not (0 <= w < W):
                continue
            dxi = 4 - j
            for dy in range(5):
                if dy == 4:
                    if dr != 0:
                        continue
                    slot = 2
                elif dy % 2 == dr:
                    slot = (dy - dr) // 2
                else:
                    continue
                for p in range(4):
                    i, jj = p // 2, p % 2
                    ch = p * 25 + dy * 5 + dxi  # p-major enc channels
                    si3[q, j * E + ch] = slot * 256 + i * 128 + w * 2 + jj
    return si3


def _build_program():
    if "nc" in _CACHE:
        return _CACHE["nc"]

    import concourse.bacc as bacc
    import concourse.mybir as mybir
    import concourse.tile as tile
    from concourse import bass

    F32, F16, I16 = mybir.dt.float32, mybir.dt.float16, mybir.dt.int16
    PSUM = bass.MemorySpace.PSUM
    Act = mybir.ActivationFunctionType

    nc = bacc.Bacc("TRN2", target_bir_lowering=False, debug=False, num_devices=8)

    xs_d = nc.dram_tensor("xs", [2, 128, RS, WP], F16, kind="ExternalInput")
    xp_d = nc.dram_tensor("xp", [2, 128, RS - 1, 128], F16, kind="ExternalInput")
    ba_d = nc.dram_tensor("blobA", [128, 2 * CM], F16, kind="ExternalInput")
    sa_d = nc.dram_tensor("smallA", [1, CM + RS * WP], F16, kind="ExternalInput")
    bw_d = nc.dram_tensor("blobW", [128, 900], F16, kind="ExternalInput")
    bo_d = nc.dram_tensor("blobO", [128, 2 * OC], F16, kind="ExternalInput")
    bc_d = nc.dram_tensor("blobC", [128, 130], F32, kind="ExternalInput")
    si_d = nc.dram_tensor("six", [128, 512], I16, kind="ExternalInput")
    out_d = nc.dram_tensor("out", [2, 128, HH, 2, 128], F32, kind="ExternalOutput")

    with tile.TileContext(nc) as tc:
        with (
            tc.tile_pool(name="const", bufs=1) as cp,
            tc.tile_pool(name="esb", bufs=3) as ep_sb,
            tc.tile_pool(name="sm", bufs=8) as smp,
            tc.tile_pool(name="sB", bufs=6) as bp,
            tc.tile_pool(name="ro", bufs=4) as rop,
        ):
            xs0 = cp.tile([128, RS, WP], F16, tag="xs0")
            xs1 = cp.tile([128, RS, WP], F16, tag="xs1")
            xp0 = cp.tile([128, RS - 1, 128], F16, tag="xp0")
            xp1 = cp.tile([128, RS - 1, 128], F16, tag="xp1")
            ba_t = cp.tile([128, 2 * CM], F16, tag="blobA")
            sa_t = cp.tile([1, CM + RS * WP], F16, tag="smallA")
            bw_t = cp.tile([128, 900], F16, tag="blobW")
            bo_t = cp.tile([128, 2 * OC], F16, tag="blobO")
            bc_t = cp.tile([128, 130], F32, tag="blobC")
            si_t = cp.tile([128, 512], I16, tag="six")
            t_t = cp.tile([CM + 1, TR, WP], F16, tag="t")
            kern = cp.tile([128, 16, E], F16, tag="kern")
            S3 = cp.tile([128, 2, 16, 512], F16, tag="S3")
            YS2 = cp.tile([128, RS, OC], F16, tag="YS2")

            wd0, wd1 = ba_t[:, 0:CM], ba_t[:, CM : 2 * CM]
            bd_v = sa_t[:, 0:CM]
            vm_v = sa_t[:, CM:].rearrange("p (r w) -> p r w", r=RS)
            we_v = bw_t[0 : CM + 1, :].rearrange("p (t e) -> p t e", t=9)
            wo0, wo1 = bo_t[:, 0:OC], bo_t[:, OC : 2 * OC]
            id_v = bc_t[0:E, 0:E]
            bo0, bo1 = bc_t[:, 128:129], bc_t[:, 129:130]

            # SP queue: phase-A inputs first (x slab in 3 row chunks so phase
            # A starts as soon as the first rows land).  Act queue: only the
            # immediately-needed weights early — si/wo follow the first conv
            # chunk so their transfers don't delay the x slab.
            nc.sync.dma_start(ba_t[:], ba_d[:])
            nc.sync.dma_start(sa_t[:], sa_d[:])
            r0, r1 = _XCHUNKS[0]
            nc.sync.dma_start(xs0[:, r0:r1, :], xs_d[0, :, r0:r1, :])
            nc.sync.dma_start(xs1[:, r0:r1, :], xs_d[1, :, r0:r1, :])
            nc.sync.dma_start(bw_t[:], bw_d[:])
            nc.sync.dma_start(bc_t[:], bc_d[:])
            nc.sync.dma_start(si_t[:], si_d[:])
            for r0, r1 in _XCHUNKS[1:]:
                nc.sync.dma_start(xs0[:, r0:r1, :], xs_d[0, :, r0:r1, :])
                nc.sync.dma_start(xs1[:, r0:r1, :], xs_d[1, :, r0:r1, :])
            nc.sync.dma_start(bo_t[:], bo_d[:])
            nc.sync.dma_start(xp0[:], xp_d[0])
            nc.sync.dma_start(xp1[:], xp_d[1])
            nc.vector.memset(t_t[CM : CM + 1, :, :], 1.0)
            # zero-fill S3 once on the (otherwise idle) Pool engine so the
            # j-range edge cells the scatters read are defined; split in two
            # so neither S3 batch waits on the later fill
            nc.gpsimd.memset(S3[:, :, 0:4, :], 0.0)
            nc.gpsimd.memset(S3[:, :, 4:16, :], 0.0)

            # ---- phases A+B interleaved: B chunk k needs only A chunks
            # <= k+1, so emitting A0,A1,B0,A2,B1,... gets kern chunk 0 (and
            # with it the phase-D scatter chain) started ~7us earlier than
            # a strict A-then-B order.
            with (
                tc.tile_pool(name="tp", bufs=2, space=PSUM) as tpp,
                tc.tile_pool(name="ep", bufs=2, space=PSUM) as epp,
                tc.tile_pool(name="etp", bufs=2, space=PSUM) as etpp,
            ):
                def a_chunk(r0):
                    nr = min(7, TR - r0)
                    tp = tpp.tile([CM, nr, WP], F32, tag="tp", name="tp")
                    nc.tensor.matmul(tp[:], wd0, xs0[:, 1 + r0 : 1 + r0 + nr, :],
                                     start=True, stop=False)
                    nc.tensor.matmul(tp[:], wd1, xs1[:, 1 + r0 : 1 + r0 + nr, :],
                                     start=False, stop=False)
                    nc.tensor.matmul(tp[:], bd_v, vm_v[:, 1 + r0 : 1 + r0 + nr, :],
                                     start=False, stop=True)
                    nc.vector.tensor_copy(t_t[0:CM, r0 : r0 + nr, :], tp[:])

                def b_chunk(r0, nr, s0, ns):
                    ep = epp.tile([E, nr, W], F32, tag="ep", name="ep")
                    for tap in range(9):
                        dy, dx = tap // 3, tap % 3
                        nc.tensor.matmul(
                            ep[:],
                            we_v[:, tap, :],
                            t_t[:, r0 + dy : r0 + dy + nr, 1 + dx : 1 + dx + W],
                            start=(tap == 0), stop=(tap == 8),
                        )
                    es = ep_sb.tile([E, nr, W], F32, tag="es", name="es")
                    nc.scalar.activation(es[:], ep[:], Act.Copy)
                    for s in range(ns):
                        etp = etpp.tile([128, E], F32, tag="etp", name="etp")
                        nc.tensor.transpose(etp[:], es[:, 2 * s : 2 * s + 2, :],
                                            id_v)
                        slot = kern[:, s0 + s, :]
                        nc.scalar.activation(slot, etp[:], Act.Exp)
                        kv = slot.rearrange("p (q k) -> p q k", q=4)
                        ssum = smp.tile([128, 4, 1], F32, tag="ssum", name="ssum")
                        nc.vector.tensor_reduce(ssum[:], kv, mybir.AxisListType.X,
                                                mybir.AluOpType.add)
                        rinv = smp.tile([128, 4, 1], F32, tag="rinv", name="rinv")
                        nc.vector.reciprocal(rinv[:], ssum[:])
                        nc.vector.tensor_tensor(kv, kv, rinv[:].to_broadcast([128, 4, 25]),
                                                mybir.AluOpType.mult)
                def s3_batch(s0, ns, split=False):
                    # S3 fill for slots [s0, s0+ns): 5 partition-shifted kern
                    # copies + 1 dr-duplicate per parity.  Parity 0 goes first
                    # (it gates the even output rows); the first batch's
                    # parity-1 group runs on the Act queue to shorten the
                    # scatter-critical chain.
                    for par in range(2):
                        q = nc.scalar if (split and par == 1) else nc.sync
                        for j in range(5):
                            w0, cnt, q0 = _JRANGES[j]
                            q.dma_start(
                                S3[q0 : q0 + cnt, par, s0 : s0 + ns,
                                   j * E : j * E + E],
                                kern[64 * par + w0 : 64 * par + w0 + cnt,
                                     s0 : s0 + ns, :],
                            )
                        q.dma_start(S3[64:128, par, s0 : s0 + ns, :],
                                    S3[0:64, par, s0 : s0 + ns, :])

                a_chunk(0)
                a_chunk(7)
                a_chunk(14)
                a_chunk(21)
                a_chunk(28)
                b_chunk(0, 8, 0, 4)
                s3_batch(0, 4, split=True)
                b_chunk(8, 8, 4, 4)
                s3_batch(4, 4)
                b_chunk(16, 8, 8, 4)
                s3_batch(8, 4)
                b_chunk(24, 8, 12, 4)
                s3_batch(12, 4)

            # ---- phases C+D interleaved ----
            # C: YS2[(dr,w'), g] = y0[row g-2+dr, col w'] fp16; rows beyond
            # g=4 are emitted inside the D loop (D row h needs g <= h+4).
            # D: banded reassembly, 3 matmuls per (h, c-half).
            with (
                tc.tile_pool(name="yp", bufs=2, space=PSUM) as ypp,
                tc.tile_pool(name="rp", bufs=4, space=PSUM) as rpp,
            ):
                def c_row(g):
                    yp = ypp.tile([128, OC], F32, tag="yp", name="yp")
                    nc.tensor.matmul(yp[:], xp0[:, g, :], wo0,
                                     start=True, stop=False)
                    nc.tensor.matmul(yp[:], xp1[:, g, :], wo1,
                                     start=False, stop=True)
                    nc.scalar.activation(YS2[:, g, :], yp[:], Act.Copy)

                for g in range(5):
                    c_row(g)
                # process rows even-ahead (0, 2, 1, 4, 3, ...): even rows are
                # gated only on the parity-0 S3 stream, keeping Pool busy
                # while each batch's parity-1 DMAs land.
                OB = 4          # output rows per DMA batch
                order = [0] + [x for k in range(1, HH // 2)
                               for x in (2 * k, 2 * k - 1)] + [HH - 1]
                robs = {}
                done = [0] * (HH // OB)
                next_c = 5
                for h in order:
                    b0 = h - h % OB
                    if b0 not in robs:
                        robs[b0] = (
                            rop.tile([128, OB, 2, 128], F32, tag="rob0",
                                     name="rob0"),
                            rop.tile([128, OB, 2, 128], F32, tag="rob1",
                                     name="rob1"),
                        )
                    rob = robs[b0]
                    Bc = bp.tile([128, 768], F16, tag="Bc")
                    nc.gpsimd.local_scatter(Bc[:], S3[:, h % 2, h // 2, :], si_t[:],
                                            channels=128, num_elems=768, num_idxs=512)
                    while next_c <= min(h + 6, RS - 2):
                        c_row(next_c)
                        next_c += 1
                    for cf in range(2):
                        rp = rpp.tile([128, 2, 128], F32, tag="rp")
                        nc.tensor.matmul(rp[:], YS2[:, h, 128 * cf : 128 * (cf + 1)],
                                         Bc[:, 0:256], start=True, stop=False)
                        nc.tensor.matmul(rp[:], YS2[:, h + 2, 128 * cf : 128 * (cf + 1)],
                                         Bc[:, 256:512], start=False, stop=False)
                        nc.tensor.matmul(rp[:], YS2[0:64, h + 4, 128 * cf : 128 * (cf + 1)],
                                         Bc[0:64, 512:768], start=False, stop=True)
                        dst = rob[cf][:, h % OB, :, :]
                        if cf == 0:
                            nc.vector.tensor_tensor(dst, rp[:],
                                                    bo0.to_broadcast([128, 2, 128]),
                                                    mybir.AluOpType.add)
                        else:
                            nc.scalar.activation(dst, rp[:], Act.Identity,
                                                 bias=bo1)
                    done[b0 // OB] += 1
                    if done[b0 // OB] == OB:
                        nc.scalar.dma_start(out_d[0, :, b0 : b0 + OB, :, :],
                                            rob[0][:])
                        nc.scalar.dma_start(out_d[1, :, b0 : b0 + OB, :, :],
                                            rob[1][:])
                        del robs[b0]

    nc.compile()
    _CACHE["nc"] = nc
    return nc


def _host_inputs(x, W_down, b_down, W_enc, b_enc, W_out, b_out):
    """Per-core input maps (core = 2*n + h_half)."""
    blobA = np.ascontiguousarray(
        W_down.T.reshape(2, 128, CM).transpose(1, 0, 2).reshape(128, 2 * CM),
        np.float16)
    # p-major enc-channel permutation: ch' = p*25 + k  (orig ch = k*4 + p)
    perm = np.array([k * 4 + p for p in range(4) for k in range(25)])
    we = np.zeros((128, 9, E), np.float16)
    for tap in range(9):
        dy, dx = tap // 3, tap % 3
        we[:CM, tap, :] = W_enc[perm, :, dy, dx].T.astype(np.float16)
    we[CM, 4, :] = b_enc[perm].astype(np.float16)
    blobW = we.reshape(128, 900)
    blobO = np.ascontiguousarray(
        W_out.T.reshape(2, 128, OC).transpose(1, 0, 2).reshape(128, 2 * OC),
        np.float16)
    blobC = np.concatenate(
        [np.eye(128, dtype=np.float32), b_out.reshape(2, 128).T.astype(np.float32)],
        axis=1)
    six = _scatter_index_table()

    in_maps = []
    for core in range(8):
        n, h0 = core // 2, (core % 2) * HH
        xs = np.zeros((C, RS, WP), np.float16)
        vm = np.zeros((RS, WP), np.float16)
        lo, hi = max(0, h0 - 2), min(H, h0 + HH + 2)
        xs[:, lo - (h0 - 2) : hi - (h0 - 2), 2 : 2 + W] = x[n, :, lo:hi, :]
        vm[lo - (h0 - 2) : hi - (h0 - 2), 2 : 2 + W] = 1.0
        smallA = np.concatenate(
            [b_down.astype(np.float16), vm.reshape(-1)])[None, :].astype(np.float16)
        # xp: phase-C stationary pairs xp[c, g, rr*64+w] = xs[c, g+rr, 2+w]
        sl = xs[:, :, 2 : 2 + W]
        xp = np.ascontiguousarray(
            np.lib.stride_tricks.sliding_window_view(sl, 2, axis=1)
            .transpose(0, 1, 3, 2).reshape(C, RS - 1, 128), np.float16)
        in_maps.append({
            "xs": xs.reshape(2, 128, RS, WP),
            "xp": xp.reshape(2, 128, RS - 1, 128),
            "blobA": blobA, "smallA": smallA, "blobW": blobW, "blobO": blobO,
            "blobC": blobC, "six": six,
        })
    return in_maps


def kernel(x, W_down, b_down, W_enc, b_enc, W_out, b_out):
    from concourse.bass_utils import run_bass_kernel_spmd

    nc = _build_program()
    in_maps = _host_inputs(np.asarray(x, np.float32), np.asarray(W_down, np.float32),
                           np.asarray(b_down, np.float32), np.asarray(W_enc, np.float32),
                           np.asarray(b_enc, np.float32), np.asarray(W_out, np.float32),
                           np.asarray(b_out, np.float32))
    res = run_bass_kernel_spmd(nc, in_maps, list(range(8)))
    full = np.empty((N, C, 2 * H, 2 * W), np.float32)
    for core in range(8):
        n, half = core // 2, core % 2
        arr = res.results[core]["out"].reshape(C, HH * 2, 2 * W)
        full[n, :, half * 64 : (half + 1) * 64, :] = arr
    return full


# revision 74
# speedup vs baseline: 1.1531x; 1.1259x over previous
"""CARAFE content-aware upsampling kernel for Trainium2 (8 NeuronCores).

Problem: x(4,256,64,64) -> 1x1 down-conv(64ch) -> 3x3 enc-conv(100ch) ->
softmax over 25 reassembly taps -> content-aware reassembly + pixel shuffle
(x2) -> 1x1 out-conv(256ch).  Output (4,256,128,128).

Sharding: data-parallel over (batch n, H-half) = 8 shards; each core computes
32 output rows (64 upsampled rows) of one image.

Per-core algorithm (v4 — fp16 matmuls, minimal DMA count, dy-pair packing):
  A) t = W_down@x + b_down          (64, 34, 68)  channels-on-partitions
  B) e = conv3x3(t) + b_enc         (100, 32*64)  via 9 shifted fp16 matmuls,
     PE-transpose (w-interleaved) -> softmax over 25 taps -> kern fp16 with
     partitions (w*2 + row-parity); 6 plain-slice DMAs per chunk build the
     partition-shifted S3 operand for phase D's scatters.
  C) y0 = W_out@x (bias added post-reassembly; exact because the softmax
     weights sum to 1 and zero-padded x gives y0=0 at pad positions).
     Stationary = xs[:, g:g+2, 2:66] so PSUM partitions come out as
     (row-offset dr, col w') = the layout phase D needs (YS2).  Rows are
     emitted interleaved with phase D to keep PE warm while scatters run.
  D) reassembly per output row h: one gpsimd local_scatter builds a banded
     fp16 matrix Bc[(dr,w'), (slot,i,w,jj)] packing dy-pairs {0,1},{2,3} into
     128-partition contractions plus a 64-partition dy=4 tile -> 3
     PSUM-accumulated matmuls per c-half.  b_out is added during the
     PSUM->SBUF copy (DVE for c-half 0, Act for c-half 1), 4 rows per output
     DMA (cf0 on the SP queue, cf1 on the Act queue).
"""
import sys

for _p in ("/opt/trn_rl_repo",):
    if _p not in sys.path:
        sys.path.insert(0, _p)

import numpy as np

N, C, H, W = 4, 256, 64, 64
D, KUP = 2, 5
CM, E, OC = 64, 100, 256
HH = 32          # output rows per core
RS = 37          # x slab rows (2-halo each side + 1 pad row for phase C pairs)
TR = HH + 2      # t rows (1-halo each side)
WP = W + 4       # padded width

_CACHE = {}

# per-j valid-w windows for the S3 partition-shifted copies:
# S3[q, par, s, j*100+ch] = kern[2*w + par, s, ch] with w = q%64 + j - 2
_JRANGES = [(0, 62, 2), (0, 63, 1), (0, 64, 0), (1, 63, 0), (2, 62, 0)]

# x slab DMA row chunks (phase A starts once the first chunk lands)
_XCHUNKS = ((0, 8), (8, 22), (22, RS))


def _scatter_index_table() -> np.ndarray:
    """si3[q, j*100+ch] -> column in banded Bc[128, 768].

    Partition q = dr*64 + w' (dr = dy-pair row offset, w' = y column).
    Bc columns: slot*256 + i*128 + w*2 + jj, slot 0 = dy{0,1}, slot 1 =
    dy{2,3}, slot 2 = dy 4 (dr=0 partitions only).
    """
    si3 = np.full((128, 512), -1, np.int16)
    for q in range(128):
        dr, wpp = q // 64, q % 64
        for j in range(5):
            w = wpp + j - 2
            if not (0 <= w < W):
                continue
            dxi = 4 - j
            for dy in range(5):
                if dy == 4:
                    if dr != 0:
                        continue
                    slot = 2
                elif dy % 2 == dr:
                    slot = (dy - dr) // 2
                else:
                    continue
                for p in range(4):
                    i, jj = p // 2, p % 2
                    ch = p * 25 + dy * 5 + dxi  # p-major enc channels
                    si3[q, j * E + ch] = slot * 256 + i * 128 + w * 2 + jj
    return si3


def _build_program():
    if "nc" in _CACHE:
        return _CACHE["nc"]

    import concourse.bacc as bacc
    import concourse.mybir as mybir
    import concourse.tile as tile
    from concourse import bass

    F32, F16, I16 = mybir.dt.float32, mybir.dt.float16, mybir.dt.int16
    PSUM = bass.MemorySpace.PSUM
    Act = mybir.ActivationFunctionType

    nc = bacc.Bacc("TRN2", target_bir_lowering=False, debug=False, num_devices=8)

    xs_d = nc.dram_tensor("xs", [2, 128, RS, WP], F16, kind="ExternalInput")
    xp_d = nc.dram_tensor("xp", [2, 128, RS - 1, 128], F16, kind="ExternalInput")
    ba_d = nc.dram_tensor("blobA", [128, 2 * CM], F16, kind="ExternalInput")
    sa_d = nc.dram_tensor("smallA", [1, CM + RS * WP], F16, kind="ExternalInput")
    bw_d = nc.dram_tensor("blobW", [128, 900], F16, kind="ExternalInput")
    bo_d = nc.dram_tensor("blobO", [128, 2 * OC], F16, kind="ExternalInput")
    bc_d = nc.dram_tensor("blobC", [128, 130], F32, kind="ExternalInput")
    si_d = nc.dram_tensor("six", [128, 512], I16, kind="ExternalInput")
    out_d = nc.dram_tensor("out", [2, 128, HH, 2, 128], F32, kind="ExternalOutput")

    with tile.TileContext(nc) as tc:
        with (
            tc.tile_pool(name="const", bufs=1) as cp,
            tc.tile_pool(name="esb", bufs=3) as ep_sb,
            tc.tile_pool(name="sm", bufs=8) as smp,
            tc.tile_pool(name="sB", bufs=6) as bp,
            tc.tile_pool(name="ro", bufs=4) as rop,
        ):
            xs0 = cp.tile([128, RS, WP], F16, tag="xs0")
            xs1 = cp.tile([128, RS, WP], F16, tag="xs1")
            xp0 = cp.tile([128, RS - 1, 128], F16, tag="xp0")
            xp1 = cp.tile([128, RS - 1, 128], F16, tag="xp1")
            ba_t = cp.tile([128, 2 * CM], F16, tag="blobA")
            sa_t = cp.tile([1, CM + RS * WP], F16, tag="smallA")
            bw_t = cp.tile([128, 900], F16, tag="blobW")
            bo_t = cp.tile([128, 2 * OC], F16, tag="blobO")
            bc_t = cp.tile([128, 130], F32, tag="blobC")
            si_t = cp.tile([128, 512], I16, tag="six")
            t_t = cp.tile([CM + 1, TR, WP], F16, tag="t")
            kern = cp.tile([128, 16, E], F16, tag="kern")
            S3 = cp.tile([128, 2, 16, 512], F16, tag="S3")
            YS2 = cp.tile([128, RS, OC], F16, tag="YS2")

            wd0, wd1 = ba_t[:, 0:CM], ba_t[:, CM : 2 * CM]
            bd_v = sa_t[:, 0:CM]
            vm_v = sa_t[:, CM:].rearrange("p (r w) -> p r w", r=RS)
            we_v = bw_t[0 : CM + 1, :].rearrange("p (t e) -> p t e", t=9)
            wo0, wo1 = bo_t[:, 0:OC], bo_t[:, OC : 2 * OC]
            id_v = bc_t[0:E, 0:E]
            bo0, bo1 = bc_t[:, 128:129], bc_t[:, 129:130]

            # SP queue: phase-A inputs first (x slab in 3 row chunks so phase
            # A starts as soon as the first rows land).  Act queue: only the
            # immediately-needed weights early — si/wo follow the first conv
            # chunk so their transfers don't delay the x slab.
            nc.sync.dma_start(ba_t[:], ba_d[:])
            nc.sync.dma_start(sa_t[:], sa_d[:])
            r0, r1 = _XCHUNKS[0]
            nc.sync.dma_start(xs0[:, r0:r1, :], xs_d[0, :, r0:r1, :])
            nc.sync.dma_start(xs1[:, r0:r1, :], xs_d[1, :, r0:r1, :])
            nc.sync.dma_start(bw_t[:], bw_d[:])
            nc.sync.dma_start(bc_t[:], bc_d[:])
            nc.sync.dma_start(si_t[:], si_d[:])
            for r0, r1 in _XCHUNKS[1:]:
                nc.sync.dma_start(xs0[:, r0:r1, :], xs_d[0, :, r0:r1, :])
                nc.sync.dma_start(xs1[:, r0:r1, :], xs_d[1, :, r0:r1, :])
            nc.sync.dma_start(bo_t[:], bo_d[:])
            nc.sync.dma_start(xp0[:], xp_d[0])
            nc.sync.dma_start(xp1[:], xp_d[1])
            nc.vector.memset(t_t[CM : CM + 1, :, :], 1.0)
            # zero-fill S3 once on the (otherwise idle) Pool engine so the
            # j-range edge cells the scatters read are defined; split in two
            # so neither S3 batch waits on the later fill
            nc.gpsimd.memset(S3[:, :, 0:4, :], 0.0)
            nc.gpsimd.memset(S3[:, :, 4:16, :], 0.0)

            # ---- phases A+B interleaved: B chunk k needs only A chunks
            # <= k+1, so emitting A0,A1,B0,A2,B1,... gets kern chunk 0 (and
            # with it the phase-D scatter chain) started ~7us earlier than
            # a strict A-then-B order.
            with (
                tc.tile_pool(name="tp", bufs=2, space=PSUM) as tpp,
                tc.tile_pool(name="ep", bufs=2, space=PSUM) as epp,
                tc.tile_pool(name="etp", bufs=2, space=PSUM) as etpp,
            ):
                def a_chunk(r0):
                    nr = min(7, TR - r0)
                    tp = tpp.tile([CM, nr, WP], F32, tag="tp", name="tp")
                    nc.tensor.matmul(tp[:], wd0, xs0[:, 1 + r0 : 1 + r0 + nr, :],
                                     start=True, stop=False)
                    nc.tensor.matmul(tp[:], wd1, xs1[:, 1 + r0 : 1 + r0 + nr, :],
                                     start=False, stop=False)
                    nc.tensor.matmul(tp[:], bd_v, vm_v[:, 1 + r0 : 1 + r0 + nr, :],
                                     start=False, stop=True)
                    nc.vector.tensor_copy(t_t[0:CM, r0 : r0 + nr, :], tp[:])

                def b_chunk(r0, nr, s0, ns):
                    ep = epp.tile([E, nr, W], F32, tag="ep", name="ep")
                    for tap in range(9):
                        dy, dx = tap // 3, tap % 3
                        nc.tensor.matmul(
                            ep[:],
                            we_v[:, tap, :],
                            t_t[:, r0 + dy : r0 + dy + nr, 1 + dx : 1 + dx + W],
                            start=(tap == 0), stop=(tap == 8),
                        )
                    es = ep_sb.tile([E, nr, W], F32, tag="es", name="es")
                    nc.scalar.activation(es[:], ep[:], Act.Copy)
                    for s in range(ns):
                        etp = etpp.tile([128, E], F32, tag="etp", name="etp")
                        nc.tensor.transpose(etp[:], es[:, 2 * s : 2 * s + 2, :],
                                            id_v)
                        slot = kern[:, s0 + s, :]
                        nc.scalar.activation(slot, etp[:], Act.Exp)
                        kv = slot.rearrange("p (q k) -> p q k", q=4)
                        ssum = smp.tile([128, 4, 1], F32, tag="ssum", name="ssum")
                        nc.vector.tensor_reduce(ssum[:], kv, mybir.AxisListType.X,
                                                mybir.AluOpType.add)
                        rinv = smp.tile([128, 4, 1], F32, tag="rinv", name="rinv")
                        nc.vector.reciprocal(rinv[:], ssum[:])
                        nc.vector.tensor_tensor(kv, kv, rinv[:].to_broadcast([128, 4, 25]),
                                                mybir.AluOpType.mult)
                def s3_batch(s0, ns, split=False):
                    # S3 fill for slots [s0, s0+ns): 5 partition-shifted kern
                    # copies + 1 dr-duplicate per parity.  Parity 0 goes first
                    # (it gates the even output rows); the first batch's
                    # parity-1 group runs on the Act queue to shorten the
                    # scatter-critical chain.
                    for par in range(2):
                        q = nc.scalar if (split and par == 1) else nc.sync
                        for j in range(5):
                            w0, cnt, q0 = _JRANGES[j]
                            q.dma_start(
                                S3[q0 : q0 + cnt, par, s0 : s0 + ns,
                                   j * E : j * E + E],
                                kern[64 * par + w0 : 64 * par + w0 + cnt,
                                     s0 : s0 + ns, :],
                            )
                        q.dma_start(S3[64:128, par, s0 : s0 + ns, :],
                                    S3[0:64, par, s0 : s0 + ns, :])

                a_chunk(0)
                a_chunk(7)
                b_chunk(0, 8, 0, 4)
                s3_batch(0, 4, split=True)
                a_chunk(14)
                b_chunk(8, 8, 4, 4)
                s3_batch(4, 4)
                a_chunk(21)
                b_chunk(16, 8, 8, 4)
                s3_batch(8, 4)
                a_chunk(28)
                b_chunk(24, 8, 12, 4)
                s3_batch(12, 4)

            # ---- phases C+D interleaved ----
            # C: YS2[(dr,w'), g] = y0[row g-2+dr, col w'] fp16; rows beyond
            # g=4 are emitted inside the D loop (D row h needs g <= h+4).
            # D: banded reassembly, 3 matmuls per (h, c-half).
            with (
                tc.tile_pool(name="yp", bufs=2, space=PSUM) as ypp,
                tc.tile_pool(name="rp", bufs=4, space=PSUM) as rpp,
            ):
                def c_row(g):
                    yp = ypp.tile([128, OC], F32, tag="yp", name="yp")
                    nc.tensor.matmul(yp[:], xp0[:, g, :], wo0,
                                     start=True, stop=False)
                    nc.tensor.matmul(yp[:], xp1[:, g, :], wo1,
                                     start=False, stop=True)
                    nc.scalar.activation(YS2[:, g, :], yp[:], Act.Copy)

                for g in range(5):
                    c_row(g)
                # process rows even-ahead (0, 2, 1, 4, 3, ...): even rows are
                # gated only on the parity-0 S3 stream, keeping Pool busy
                # while each batch's parity-1 DMAs land.
                OB = 4          # output rows per DMA batch
                order = [0] + [x for k in range(1, HH // 2)
                               for x in (2 * k, 2 * k - 1)] + [HH - 1]
                robs = {}
                done = [0] * (HH // OB)
                next_c = 5
                for h in order:
                    b0 = h - h % OB
                    if b0 not in robs:
                        robs[b0] = (
                            rop.tile([128, OB, 2, 128], F32, tag="rob0",
                                     name="rob0"),
                            rop.tile([128, OB, 2, 128], F32, tag="rob1",
                                     name="rob1"),
                        )
                    rob = robs[b0]
                    Bc = bp.tile([128, 768], F16, tag="Bc")
                    nc.gpsimd.local_scatter(Bc[:], S3[:, h % 2, h // 2, :], si_t[:],
                                            channels=128, num_elems=768, num_idxs=512)
                    while next_c <= min(h + 6, RS - 2):
                        c_row(next_c)
                        next_c += 1
                    for cf in range(2):
                        rp = rpp.tile([128, 2, 128], F32, tag="rp")
                        nc.tensor.matmul(rp[:], YS2[:, h, 128 * cf : 128 * (cf + 1)],
                                         Bc[:, 0:256], start=True, stop=False)
                        nc.tensor.matmul(rp[:], YS2[:, h + 2, 128 * cf : 128 * (cf + 1)],
                                         Bc[:, 256:512], start=False, stop=False)
                        nc.tensor.matmul(rp[:], YS2[0:64, h + 4, 128 * cf : 128 * (cf + 1)],
                                         Bc[0:64, 512:768], start=False, stop=True)
                        dst = rob[cf][:, h % OB, :, :]
                        if cf == 0:
                            nc.vector.tensor_tensor(dst, rp[:],
                                                    bo0.to_broadcast([128, 2, 128]),
                                                    mybir.AluOpType.add)
                        else:
                            nc.scalar.activation(dst, rp[:], Act.Identity,
                                                 bias=bo1)
                    done[b0 // OB] += 1
                    if done[b0 // OB] == OB:
                        nc.scalar.dma_start(out_d[0, :, b0 : b0 + OB, :, :],
                                            rob[0][:])
                        nc.scalar.dma_start(out_d[1, :, b0 : b0 + OB, :, :],
                                            rob[1][:])
                        del robs[b0]

    nc.compile()
    _CACHE["nc"] = nc
    return nc


def _host_inputs(x, W_down, b_down, W_enc, b_enc, W_out, b_out):
    """Per-core input maps (core = 2*n + h_half)."""
    blobA = np.ascontiguousarray(
        W_down.T.reshape(2, 128, CM).transpose(1, 0, 2).reshape(128, 2 * CM),
        np.float16)
    # p-major enc-channel permutation: ch' = p*25 + k  (orig ch = k*4 + p)
    perm = np.array([k * 4 + p for p in range(4) for k in range(25)])
    we = np.zeros((128, 9, E), np.float16)
    for tap in range(9):
        dy, dx = tap // 3, tap % 3
        we[:CM, tap, :] = W_enc[perm, :, dy, dx].T.astype(np.float16)
    we[CM, 4, :] = b_enc[perm].astype(np.float16)
    blobW = we.reshape(128, 900)
    blobO = np.ascontiguousarray(
        W_out.T.reshape(2, 128, OC).transpose(1, 0, 2).reshape(128, 2 * OC),
        np.float16)
    blobC = np.concatenate(
        [np.eye(128, dtype=np.float32), b_out.reshape(2, 128).T.astype(np.float32)],
        axis=1)
    six = _scatter_index_table()

    in_maps = []
    for core in range(8):
        n, h0 = core // 2, (core % 2) * HH
        xs = np.zeros((C, RS, WP), np.float16)
        vm = np.zeros((RS, WP), np.float16)
        lo, hi = max(0, h0 - 2), min(H, h0 + HH + 2)
        xs[:, lo - (h0 - 2) : hi - (h0 - 2), 2 : 2 + W] = x[n, :, lo:hi, :]
        vm[lo - (h0 - 2) : hi - (h0 - 2), 2 : 2 + W] = 1.0
        smallA = np.concatenate(
            [b_down.astype(np.float16), vm.reshape(-1)])[None, :].astype(np.float16)
        # xp: phase-C stationary pairs xp[c, g, rr*64+w] = xs[c, g+rr, 2+w]
        sl = xs[:, :, 2 : 2 + W]
        xp = np.ascontiguousarray(
            np.lib.stride_tricks.sliding_window_view(sl, 2, axis=1)
            .transpose(0, 1, 3, 2).reshape(C, RS - 1, 128), np.float16)
        in_maps.append({
            "xs": xs.reshape(2, 128, RS, WP),
            "xp": xp.reshape(2, 128, RS - 1, 128),
            "blobA": blobA, "smallA": smallA, "blobW": blobW, "blobO": blobO,
            "blobC": blobC, "six": six,
        })
    return in_maps


def kernel(x, W_down, b_down, W_enc, b_enc, W_out, b_out):
    from concourse.bass_utils import run_bass_kernel_spmd

    nc = _build_program()
    in_maps = _host_inputs(np.asarray(x, np.float32), np.asarray(W_down, np.float32),
                           np.asarray(b_down, np.float32), np.asarray(W_enc, np.float32),
                           np.asarray(b_enc, np.float32), np.asarray(W_out, np.float32),
                           np.asarray(b_out, np.float32))
    res = run_bass_kernel_spmd(nc, in_maps, list(range(8)))
    full = np.empty((N, C, 2 * H, 2 * W), np.float32)
    for core in range(8):
        n, half = core // 2, core % 2
        arr = res.results[core]["out"].reshape(C, HH * 2, 2 * W)
        full[n, :, half * 64 : (half + 1) * 64, :] = arr
    return full
